# revision 1
# baseline (speedup 1.0000x reference)
"""CrossCoder kernel for 8 Trainium2 NeuronCores (Bass/Tile, SPMD).

Math (reference):
    f     = relu(einsum('bld,ldf->bf', x, W_enc) + b_enc)     # [B, F]
    x_hat = einsum('bf,lfd->bld', f, W_dec) + b_dec           # [B, L, D]

Sharding: dict dim F=32768 split 8 ways (FL=4096 per core, tensor parallel
over latents). Each core computes its local f shard (encode) and the
partial decode sum over its latents; ReduceScatters combine the partials,
leaving each core with a distinct slice of the (LD=2048, B) transposed
output, which the host reassembles and transposes back.

Device layout is feature-major (contraction dim on SBUF partitions); batch
runs in two halves of 512 inside ONE TileContext. Collectives are emitted
in-context: RS0 after half 0 overlaps all of half 1; half 1's partial is
split in two (ld rows 0-1023 / 1024-2047) so RS1a overlaps the tail of the
decode and only RS1b (2MB) is exposed. Weights/x are host-repacked into
contiguous [128, 512] tiles so every DMA is one 256KB contiguous block.
b_dec/8 is folded in pre-collective. All matmuls are float32r (full PE
rate, ~2e-4 rel err).
"""

import numpy as np

B = 1024
L = 2
D = 1024
F = 32768
NCORES = 8
FL = F // NCORES      # 4096 latents per core
LD = L * D            # 2048
KT = LD // 128        # 16 encode k-tiles
FT = FL // 128        # 32 f-tiles per core
NB = 512              # matmul moving free dim
NH = 2                # batch halves

_CACHE = {}


def _build_nc():
    import concourse.bass as bass  # noqa: F401
    import concourse.tile as tile
    from concourse import bacc, mybir

    f32 = mybir.dt.float32
    f32r = mybir.dt.float32r

    nc = bacc.Bacc()

    xT = nc.declare_dram_parameter("xT", [NH, KT, 128, NB], f32r, isOutput=False)
    w_enc = nc.declare_dram_parameter("w_enc", [KT, FT // 4, 128, NB], f32r, isOutput=False)
    w_dec = nc.declare_dram_parameter("w_dec", [L, 2, FT, 128, NB], f32r, isOutput=False)
    b_enc = nc.declare_dram_parameter("b_enc", [128, FT], f32, isOutput=False)
    b_dec8 = nc.declare_dram_parameter("b_dec8", [128, KT], f32, isOutput=False)
    # out_sh: [0:2] = h0 ld-tiles {2i,2i+1}; [2] = h1 ld-tile i; [3] = h1 ld-tile 8+i
    out_sh = nc.declare_dram_parameter("out_sh", [4, 128, NB], f32, isOutput=True)

    # partial buffers: one per (half, l-block) so each ReduceScatter fires as
    # soon as its 8 ld-tiles are written, spreading collective traffic
    partial0 = nc.dram_tensor("partial0", [KT, 128, NB], f32)
    parts1 = [nc.dram_tensor(f"partial1{l}", [KT // 2, 128, NB], f32) for l in range(L)]
    rs0 = nc.dram_tensor("rs0", [2, 128, NB], f32)
    rss1 = [nc.dram_tensor(f"rs1{l}", [1, 128, NB], f32) for l in range(L)]

    xT_a = xT.ap()
    w_enc_a = w_enc.ap()
    w_dec_a = w_dec.ap()
    rgroups = [list(range(NCORES))]

    with tile.TileContext(nc) as tc:
        with (
            tc.tile_pool(name="xp", bufs=1) as xp,
            tc.tile_pool(name="fp", bufs=1) as fp,
            tc.tile_pool(name="we", bufs=16) as we,
            tc.tile_pool(name="wd", bufs=16) as wd,
            tc.tile_pool(name="stg", bufs=8) as stg,
            tc.tile_pool(name="bias", bufs=1) as bias,
            tc.tile_pool(name="ps", bufs=8, space="PSUM") as ps,
        ):
            benc_t = bias.tile([128, FT], f32, name="benc")
            nc.sync.dma_start(out=benc_t, in_=b_enc.ap())
            bdec_t = bias.tile([128, KT], f32, name="bdec")
            nc.sync.dma_start(out=bdec_t, in_=b_dec8.ap())

            for h in range(NH):
                x_tiles = []
                for k in range(KT):
                    xt = xp.tile([128, NB], f32r, tag=f"x{k}", name=f"x{k}")
                    nc.sync.dma_start(out=xt, in_=xT_a[h, k])
                    x_tiles.append(xt)

                # ---- encode
                f_tiles = []
                for fg in range(FT // 4):
                    pss = [
                        ps.tile([128, NB], f32, tag="ps", name=f"pse{_j}")
                        for _j in range(4)
                    ]
                    for k in range(KT):
                        wt = we.tile([128, NB], f32r, tag="we", name="wet")
                        nc.sync.dma_start(out=wt, in_=w_enc_a[k, fg])
                        for j in range(4):
                            nc.tensor.matmul(
                                pss[j],
                                wt[:, j * 128 : (j + 1) * 128],
                                x_tiles[k],
                                start=(k == 0),
                                stop=(k == KT - 1),
                            )
                    for j in range(4):
                        ft_idx = fg * 4 + j
                        ftile = fp.tile(
                            [128, NB], f32r, tag=f"f{ft_idx}", name=f"f{ft_idx}"
                        )
                        nc.scalar.activation(
                            ftile,
                            pss[j],
                            mybir.ActivationFunctionType.Relu,
                            bias=benc_t[:, ft_idx : ft_idx + 1],
                        )
                        f_tiles.append(ftile)

                # ---- decode
                for l in range(L):
                    part_a = partial0.ap() if h == 0 else parts1[l].ap()
                    base = l * 8 if h == 0 else 0
                    for dg in range(2):
                        pss = [
                            ps.tile([128, NB], f32, tag="ps", name=f"psd{_j}")
                            for _j in range(4)
                        ]
                        for fk in range(FT):
                            wt = wd.tile([128, NB], f32r, tag="wd", name="wdt")
                            nc.sync.dma_start(out=wt, in_=w_dec_a[l, dg, fk])
                            for j in range(4):
                                nc.tensor.matmul(
                                    pss[j],
                                    wt[:, j * 128 : (j + 1) * 128],
                                    f_tiles[fk],
                                    start=(fk == 0),
                                    stop=(fk == FT - 1),
                                )
                        for j in range(4):
                            ld_t = l * 8 + dg * 4 + j
                            st = stg.tile([128, NB], f32, tag="st", name="st")
                            nc.vector.tensor_scalar_add(
                                st, pss[j], bdec_t[:, ld_t : ld_t + 1]
                            )
                            nc.sync.dma_start(
                                out=part_a[base + dg * 4 + j], in_=st
                            )
                    if h == 1:
                        # this l-block's partial is complete → ReduceScatter it
                        nc.gpsimd.collective_compute(
                            "ReduceScatter",
                            mybir.AluOpType.add,
                            ins=[parts1[l][:]],
                            outs=[rss1[l][:]],
                            replica_groups=rgroups,
                        )
                if h == 0:
                    nc.gpsimd.collective_compute(
                        "ReduceScatter",
                        mybir.AluOpType.add,
                        ins=[partial0[:]],
                        outs=[rs0[:]],
                        replica_groups=rgroups,
                    )

            out_a = out_sh.ap()
            nc.gpsimd.dma_start(out=out_a[0:2], in_=rs0[:])
            nc.gpsimd.dma_start(out=out_a[2:3], in_=rss1[0][:])
            nc.gpsimd.dma_start(out=out_a[3:4], in_=rss1[1][:])

    nc.finalize()
    return nc


def _get_nc():
    if "nc" not in _CACHE:
        _CACHE["nc"] = _build_nc()
    return _CACHE["nc"]


def kernel(x, W_enc, b_enc, W_dec, b_dec):
    from concourse.bass_utils import run_bass_kernel_spmd

    x = np.asarray(x, dtype=np.float32)
    W_enc = np.asarray(W_enc, dtype=np.float32)
    b_enc = np.asarray(b_enc, dtype=np.float32)
    W_dec = np.asarray(W_dec, dtype=np.float32)
    b_dec = np.asarray(b_dec, dtype=np.float32)

    nc = _get_nc()

    # xT blocked: [h, k, p, c] with xT row k*128+p (= x.reshape(B,LD).T), col h*512+c
    xT = np.ascontiguousarray(
        x.reshape(B, LD).T.reshape(KT, 128, NH, NB).transpose(2, 0, 1, 3)
    )
    w_enc_flat = W_enc.reshape(LD, F)
    bdec8 = np.ascontiguousarray(
        (b_dec.reshape(LD) / NCORES).astype(np.float32).reshape(KT, 128).T
    )

    in_maps = []
    for i in range(NCORES):
        fsl = slice(i * FL, (i + 1) * FL)
        we_blk = np.ascontiguousarray(
            w_enc_flat[:, fsl].reshape(KT, 128, FT // 4, NB).transpose(0, 2, 1, 3)
        )
        wd_blk = np.ascontiguousarray(
            W_dec[:, fsl, :].reshape(L, FT, 128, 2, NB).transpose(0, 3, 1, 2, 4)
        )
        in_maps.append(
            {
                "xT": xT,
                "w_enc": we_blk,
                "w_dec": wd_blk,
                "b_enc": np.ascontiguousarray(b_enc[fsl].reshape(FT, 128).T),
                "b_dec8": bdec8,
            }
        )

    res = run_bass_kernel_spmd(nc, in_maps, list(range(NCORES)))
    _CACHE["last_res"] = res

    xhatT = np.empty((LD, B), dtype=np.float32)
    for i in range(NCORES):
        arr = res.results[i]["out_sh"]  # [4, 128, NB]
        xhatT[2 * i * 128 : (2 * i + 2) * 128, 0:NB] = arr[0:2].reshape(256, NB)
        xhatT[i * 128 : (i + 1) * 128, NB : 2 * NB] = arr[2]
        xhatT[(8 + i) * 128 : (9 + i) * 128, NB : 2 * NB] = arr[3]
    return np.ascontiguousarray(xhatT.T).reshape(B, L, D).astype(np.float32)



# revision 2
# speedup vs baseline: 1.0942x; 1.0942x over previous
"""CrossCoder kernel for 8 Trainium2 NeuronCores (Bass/Tile, SPMD).

Math (reference):
    f     = relu(einsum('bld,ldf->bf', x, W_enc) + b_enc)     # [B, F]
    x_hat = einsum('bf,lfd->bld', f, W_dec) + b_dec           # [B, L, D]

Sharding: dict dim F=32768 split 8 ways (FL=4096 per core, tensor parallel
over latents). Each core computes its local f shard (encode) and the
partial decode sum over its latents; ReduceScatters combine the partials,
leaving each core with a distinct slice of the (LD=2048, B) transposed
output, which the host reassembles and transposes back.

Device layout is feature-major (contraction dim on SBUF partitions); batch
runs in two halves of 512 inside ONE TileContext. Matmul operands (x,
W_enc, W_dec, f) are bf16: full PE rate, FWL weight loads, half the HBM
traffic and PE power vs fp32r (~1e-3 rel err, tolerance 2e-2). PSUM
accumulation and the collectives stay fp32.

Collectives are emitted in-context: RS0 (h0 partial, 4MB) fires after
half 0 and overlaps all of half 1; half 1's partial is split into four
1MB (l,dg) groups, each ReduceScattered as soon as its 4 ld-tiles are
written, so only the last ~1MB RS is exposed at the tail. x for half 1
is prefetched during half 0's decode; half 0's x and first-group weights
are interleaved so the first matmul issues early.
"""

import numpy as np

B = 1024
L = 2
D = 1024
F = 32768
NCORES = 8
FL = F // NCORES      # 4096 latents per core
LD = L * D            # 2048
KT = LD // 128        # 16 encode k-tiles
FT = FL // 128        # 32 f-tiles per core
NB = 512              # matmul moving free dim
NH = 2                # batch halves

_CACHE = {}


def _build_nc():
    import concourse.bass as bass  # noqa: F401
    import concourse.tile as tile
    from concourse import bacc, mybir

    f32 = mybir.dt.float32
    bf16 = mybir.dt.bfloat16

    nc = bacc.Bacc()

    xT = nc.declare_dram_parameter("xT", [NH, KT, 128, NB], bf16, isOutput=False)
    w_enc = nc.declare_dram_parameter("w_enc", [KT, FT // 4, 128, NB], bf16, isOutput=False)
    w_dec = nc.declare_dram_parameter("w_dec", [L, 2, FT, 128, NB], bf16, isOutput=False)
    b_enc = nc.declare_dram_parameter("b_enc", [128, FT], f32, isOutput=False)
    b_dec8 = nc.declare_dram_parameter("b_dec8", [128, KT], f32, isOutput=False)
    # out_sh chunks of [64, 512]: [0:4] = h0 (rs0); [4+g] = h1 group g=(l,dg)
    out_sh = nc.declare_dram_parameter("out_sh", [8, 64, NB], f32, isOutput=True)

    # partial buffers: h0 gets one 4MB buffer (RS overlaps all of h1);
    # h1 gets one 1MB buffer per (l,dg) group so each ReduceScatter fires
    # as soon as its 4 ld-tiles are written and only the last is exposed
    partial0 = nc.dram_tensor("partial0", [KT, 128, NB], f32)
    parts1 = [nc.dram_tensor(f"partial1{g}", [4, 128, NB], f32) for g in range(4)]
    rs0 = nc.dram_tensor("rs0", [4, 64, NB], f32)
    rss1 = [nc.dram_tensor(f"rs1{g}", [1, 64, NB], f32) for g in range(4)]

    xT_a = xT.ap()
    w_enc_a = w_enc.ap()
    w_dec_a = w_dec.ap()
    rgroups = [list(range(NCORES))]

    with tile.TileContext(nc) as tc:
        with (
            tc.tile_pool(name="xp", bufs=1) as xp,
            tc.tile_pool(name="fp", bufs=1) as fp,
            tc.tile_pool(name="we", bufs=24) as we,
            tc.tile_pool(name="wd", bufs=24) as wd,
            tc.tile_pool(name="stg", bufs=8) as stg,
            tc.tile_pool(name="bias", bufs=1) as bias,
            tc.tile_pool(name="ps", bufs=8, space="PSUM") as ps,
        ):
            benc_t = bias.tile([128, FT], f32, name="benc")
            nc.sync.dma_start(out=benc_t, in_=b_enc.ap())
            bdec_t = bias.tile([128, KT], f32, name="bdec")
            nc.sync.dma_start(out=bdec_t, in_=b_dec8.ap())

            x_tiles_h = [None, None]
            for h in range(NH):
                x_tiles = x_tiles_h[h]
                if x_tiles is None:
                    x_tiles = []
                    x_tiles_h[h] = x_tiles

                # ---- encode
                f_tiles = []
                for fg in range(FT // 4):
                    pss = [
                        ps.tile([128, NB], f32, tag="ps", name=f"pse{_j}")
                        for _j in range(4)
                    ]
                    for k in range(KT):
                        if fg == 0 and h == 0:
                            # interleave x with first-group weights so the
                            # first matmul issues as early as possible
                            xt = xp.tile([128, NB], bf16, tag=f"x{h}_{k}", name=f"x{h}_{k}")
                            nc.sync.dma_start(out=xt, in_=xT_a[h, k])
                            x_tiles.append(xt)
                        wt = we.tile([128, NB], bf16, tag="we", name="wet")
                        nc.sync.dma_start(out=wt, in_=w_enc_a[k, fg])
                        for j in range(4):
                            nc.tensor.matmul(
                                pss[j],
                                wt[:, j * 128 : (j + 1) * 128],
                                x_tiles[k],
                                start=(k == 0),
                                stop=(k == KT - 1),
                            )
                    for j in range(4):
                        ft_idx = fg * 4 + j
                        ftile = fp.tile(
                            [128, NB], bf16, tag=f"f{ft_idx}", name=f"f{ft_idx}"
                        )
                        nc.scalar.activation(
                            ftile,
                            pss[j],
                            mybir.ActivationFunctionType.Relu,
                            bias=benc_t[:, ft_idx : ft_idx + 1],
                        )
                        f_tiles.append(ftile)

                if h == 0:
                    # prefetch half 1's x during half 0's decode
                    x_tiles_h[1] = []
                    for k in range(KT):
                        xt = xp.tile([128, NB], bf16, tag=f"x1_{k}", name=f"x1_{k}")
                        nc.sync.dma_start(out=xt, in_=xT_a[1, k])
                        x_tiles_h[1].append(xt)

                # ---- decode
                for l in range(L):
                    for dg in range(2):
                        g = 2 * l + dg
                        part_a = partial0.ap() if h == 0 else parts1[g].ap()
                        base = l * 8 + dg * 4 if h == 0 else 0
                        pss = [
                            ps.tile([128, NB], f32, tag="ps", name=f"psd{_j}")
                            for _j in range(4)
                        ]
                        for fk in range(FT):
                            wt = wd.tile([128, NB], bf16, tag="wd", name="wdt")
                            nc.sync.dma_start(out=wt, in_=w_dec_a[l, dg, fk])
                            for j in range(4):
                                nc.tensor.matmul(
                                    pss[j],
                                    wt[:, j * 128 : (j + 1) * 128],
                                    f_tiles[fk],
                                    start=(fk == 0),
                                    stop=(fk == FT - 1),
                                )
                        for j in range(4):
                            ld_t = l * 8 + dg * 4 + j
                            st = stg.tile([128, NB], f32, tag="st", name="st")
                            nc.vector.tensor_scalar_add(
                                st, pss[j], bdec_t[:, ld_t : ld_t + 1]
                            )
                            nc.sync.dma_start(out=part_a[base + j], in_=st)
                        if h == 1:
                            # this (l,dg) group's partial is complete → RS it
                            nc.gpsimd.collective_compute(
                                "ReduceScatter",
                                mybir.AluOpType.add,
                                ins=[parts1[g][:]],
                                outs=[rss1[g][:]],
                                replica_groups=rgroups,
                            )
                if h == 0:
                    nc.gpsimd.collective_compute(
                        "ReduceScatter",
                        mybir.AluOpType.add,
                        ins=[partial0[:]],
                        outs=[rs0[:]],
                        replica_groups=rgroups,
                    )

            out_a = out_sh.ap()
            nc.gpsimd.dma_start(out=out_a[0:4], in_=rs0[:])
            for g in range(4):
                nc.gpsimd.dma_start(out=out_a[4 + g : 5 + g], in_=rss1[g][:])

    nc.finalize()
    return nc


def _get_nc():
    if "nc" not in _CACHE:
        _CACHE["nc"] = _build_nc()
    return _CACHE["nc"]


def kernel(x, W_enc, b_enc, W_dec, b_dec):
    import ml_dtypes
    from concourse.bass_utils import run_bass_kernel_spmd

    bf16 = ml_dtypes.bfloat16
    x = np.asarray(x, dtype=np.float32)
    W_enc = np.asarray(W_enc, dtype=np.float32)
    b_enc = np.asarray(b_enc, dtype=np.float32)
    W_dec = np.asarray(W_dec, dtype=np.float32)
    b_dec = np.asarray(b_dec, dtype=np.float32)

    nc = _get_nc()

    # xT blocked: [h, k, p, c] with xT row k*128+p (= x.reshape(B,LD).T), col h*512+c
    xT = np.ascontiguousarray(
        x.reshape(B, LD).T.reshape(KT, 128, NH, NB).transpose(2, 0, 1, 3)
    ).astype(bf16)
    w_enc_flat = W_enc.reshape(LD, F)
    bdec8 = np.ascontiguousarray(
        (b_dec.reshape(LD) / NCORES).astype(np.float32).reshape(KT, 128).T
    )

    in_maps = []
    for i in range(NCORES):
        fsl = slice(i * FL, (i + 1) * FL)
        we_blk = np.ascontiguousarray(
            w_enc_flat[:, fsl].reshape(KT, 128, FT // 4, NB).transpose(0, 2, 1, 3)
        ).astype(bf16)
        wd_blk = np.ascontiguousarray(
            W_dec[:, fsl, :].reshape(L, FT, 128, 2, NB).transpose(0, 3, 1, 2, 4)
        ).astype(bf16)
        in_maps.append(
            {
                "xT": xT,
                "w_enc": we_blk,
                "w_dec": wd_blk,
                "b_enc": np.ascontiguousarray(b_enc[fsl].reshape(FT, 128).T),
                "b_dec8": bdec8,
            }
        )

    res = run_bass_kernel_spmd(nc, in_maps, list(range(NCORES)))
    _CACHE["last_res"] = res

    xhatT = np.empty((LD, B), dtype=np.float32)
    for i in range(NCORES):
        arr = res.results[i]["out_sh"]  # [8, 64, NB] fp32
        # h0: core i holds partial0 tiles {2i, 2i+1} = ld-tiles 2i,2i+1, cols 0:NB
        xhatT[2 * i * 128 : (2 * i + 2) * 128, 0:NB] = arr[0:4].reshape(256, NB)
        # h1 group g=(l,dg): core i holds ld-tile l*8+dg*4+i//2, partition
        # rows 64*(i%2) : 64*(i%2)+64, cols NB:2NB
        for g in range(4):
            l, dg = divmod(g, 2)
            ld_t = l * 8 + dg * 4 + i // 2
            r0 = ld_t * 128 + 64 * (i % 2)
            xhatT[r0 : r0 + 64, NB : 2 * NB] = arr[4 + g]
    return np.ascontiguousarray(xhatT.T).reshape(B, L, D).astype(np.float32)


# revision 3
# speedup vs baseline: 1.1098x; 1.0143x over previous
"""CrossCoder kernel for 8 Trainium2 NeuronCores (Bass/Tile, SPMD).

Math (reference):
    f     = relu(einsum('bld,ldf->bf', x, W_enc) + b_enc)     # [B, F]
    x_hat = einsum('bf,lfd->bld', f, W_dec) + b_dec           # [B, L, D]

Sharding: dict dim F=32768 split 8 ways (FL=4096 per core, tensor parallel
over latents). Each core computes its local f shard (encode) and the
partial decode sum over its latents. Cross-core reduction is done with
bf16 AllToAll + an on-core DVE tree sum (A2A moves half the bytes of a
fp32 ReduceScatter and has no CCE-reduce bottleneck), leaving each core
a distinct slice of the (LD=2048, B) transposed output which the host
reassembles.

Device layout is feature-major (contraction dim on SBUF partitions); batch
runs in two halves of 512 inside ONE TileContext. Matmul operands (x,
W_enc, W_dec, f) are bf16: full PE rate, FWL weight loads, half the HBM
traffic vs fp32r (~3e-3 rel err, tolerance 2e-2). PSUM stays fp32.

Half 0's partial (one 2MB bf16 buffer) is exchanged after half 0 and its
reduce overlaps half 1; half 1's partial is split into four 512KB (l,dg)
groups, each AllToAll'd as soon as its 4 ld-tiles are written, so only
the last small A2A + reduce is exposed at the tail. x for half 1 is
prefetched during half 0's decode; half 0's x and first-group weights
are interleaved so the first matmul issues early.
"""

import numpy as np

B = 1024
L = 2
D = 1024
F = 32768
NCORES = 8
FL = F // NCORES      # 4096 latents per core
LD = L * D            # 2048
KT = LD // 128        # 16 encode k-tiles
FT = FL // 128        # 32 f-tiles per core
NB = 512              # matmul moving free dim
NH = 2                # batch halves

_CACHE = {}


def _build_nc():
    import concourse.bass as bass  # noqa: F401
    import concourse.tile as tile
    from concourse import bacc, mybir

    f32 = mybir.dt.float32
    bf16 = mybir.dt.bfloat16
    ADD = mybir.AluOpType.add

    nc = bacc.Bacc()

    xT = nc.declare_dram_parameter("xT", [NH, KT, 128, NB], bf16, isOutput=False)
    w_enc = nc.declare_dram_parameter("w_enc", [KT, FT // 4, 128, NB], bf16, isOutput=False)
    w_dec = nc.declare_dram_parameter("w_dec", [L, 2, FT, 128, NB], bf16, isOutput=False)
    b_enc = nc.declare_dram_parameter("b_enc", [128, FT], f32, isOutput=False)
    b_dec8 = nc.declare_dram_parameter("b_dec8", [128, KT], f32, isOutput=False)
    # out rows: [0:256] = h0 ld-tiles {2i, 2i+1}; [256+64g : 320+64g] = h1
    # group g's slice (ld-tile l*8+dg*4+i//2, partition rows 64*(i%2))
    out_sh = nc.declare_dram_parameter("out_sh", [512, NB], f32, isOutput=True)

    # partial buffers (bf16): h0 one 2MB buffer; h1 one 512KB per (l,dg)
    # group so each AllToAll fires as soon as its 4 ld-tiles are written
    partial0 = nc.dram_tensor("partial0", [KT, 128, NB], bf16)
    parts1 = [nc.dram_tensor(f"partial1{g}", [4, 128, NB], bf16) for g in range(4)]
    a2a0 = nc.dram_tensor("a2a0", [8, 2, 128, NB], bf16)
    a2a1 = [nc.dram_tensor(f"a2a1{g}", [8, 64, NB], bf16) for g in range(4)]

    xT_a = xT.ap()
    w_enc_a = w_enc.ap()
    w_dec_a = w_dec.ap()
    rgroups = [list(range(NCORES))]

    with tile.TileContext(nc) as tc:
        with (
            tc.tile_pool(name="xp", bufs=1) as xp,
            tc.tile_pool(name="fp", bufs=1) as fp,
            tc.tile_pool(name="we", bufs=32) as we,
            tc.tile_pool(name="wd", bufs=32) as wd,
            tc.tile_pool(name="stg", bufs=8) as stg,
            tc.tile_pool(name="bias", bufs=1) as bias,
            tc.tile_pool(name="red", bufs=1) as red,
            tc.tile_pool(name="ps", bufs=8, space="PSUM") as ps,
        ):
            benc_t = bias.tile([128, FT], f32, name="benc")
            nc.sync.dma_start(out=benc_t, in_=b_enc.ap())
            bdec_t = bias.tile([128, KT], f32, name="bdec")
            nc.sync.dma_start(out=bdec_t, in_=b_dec8.ap())

            x_tiles_h = [None, None]
            for h in range(NH):
                x_tiles = x_tiles_h[h]
                if x_tiles is None:
                    x_tiles = []
                    x_tiles_h[h] = x_tiles

                # ---- encode
                f_tiles = []
                for fg in range(FT // 4):
                    pss = [
                        ps.tile([128, NB], f32, tag="ps", name=f"pse{_j}")
                        for _j in range(4)
                    ]
                    for k in range(KT):
                        if fg == 0 and h == 0:
                            # interleave x with first-group weights so the
                            # first matmul issues as early as possible
                            xt = xp.tile([128, NB], bf16, tag=f"x{h}_{k}", name=f"x{h}_{k}")
                            nc.sync.dma_start(out=xt, in_=xT_a[h, k])
                            x_tiles.append(xt)
                        wt = we.tile([128, NB], bf16, tag="we", name="wet")
                        nc.sync.dma_start(out=wt, in_=w_enc_a[k, fg])
                        for j in range(4):
                            nc.tensor.matmul(
                                pss[j],
                                wt[:, j * 128 : (j + 1) * 128],
                                x_tiles[k],
                                start=(k == 0),
                                stop=(k == KT - 1),
                            )
                    for j in range(4):
                        ft_idx = fg * 4 + j
                        ftile = fp.tile(
                            [128, NB], bf16, tag=f"f{ft_idx}", name=f"f{ft_idx}"
                        )
                        nc.scalar.activation(
                            ftile,
                            pss[j],
                            mybir.ActivationFunctionType.Relu,
                            bias=benc_t[:, ft_idx : ft_idx + 1],
                        )
                        f_tiles.append(ftile)

                if h == 0:
                    # prefetch half 1's x during half 0's decode
                    x_tiles_h[1] = []
                    for k in range(KT):
                        xt = xp.tile([128, NB], bf16, tag=f"x1_{k}", name=f"x1_{k}")
                        nc.sync.dma_start(out=xt, in_=xT_a[1, k])
                        x_tiles_h[1].append(xt)

                # ---- decode
                for l in range(L):
                    for dg in range(2):
                        g = 2 * l + dg
                        part_a = partial0.ap() if h == 0 else parts1[g].ap()
                        base = l * 8 + dg * 4 if h == 0 else 0
                        pss = [
                            ps.tile([128, NB], f32, tag="ps", name=f"psd{_j}")
                            for _j in range(4)
                        ]
                        for fk in range(FT):
                            wt = wd.tile([128, NB], bf16, tag="wd", name="wdt")
                            nc.sync.dma_start(out=wt, in_=w_dec_a[l, dg, fk])
                            for j in range(4):
                                nc.tensor.matmul(
                                    pss[j],
                                    wt[:, j * 128 : (j + 1) * 128],
                                    f_tiles[fk],
                                    start=(fk == 0),
                                    stop=(fk == FT - 1),
                                )
                        for j in range(4):
                            ld_t = l * 8 + dg * 4 + j
                            st = stg.tile([128, NB], bf16, tag="st", name="st")
                            nc.vector.tensor_scalar_add(
                                st, pss[j], bdec_t[:, ld_t : ld_t + 1]
                            )
                            nc.sync.dma_start(out=part_a[base + j], in_=st)
                        if h == 1:
                            # this (l,dg) group's partial is complete →
                            # exchange + on-core reduce
                            nc.gpsimd.collective_compute(
                                "AllToAll",
                                mybir.AluOpType.bypass,
                                ins=[parts1[g][:]],
                                outs=[a2a1[g][:]],
                                replica_groups=rgroups,
                            )
                            r1 = red.tile([64, 8 * NB], bf16, tag="red1", name="red1")
                            for jj in range(8):
                                nc.sync.dma_start(
                                    out=r1[:, jj * NB : (jj + 1) * NB],
                                    in_=a2a1[g].ap()[jj],
                                )
                            acc1 = red.tile([64, NB], f32, tag=f"acc1_{g}", name=f"acc1_{g}")
                            nc.vector.tensor_tensor(
                                acc1, r1[:, 0:NB], r1[:, NB : 2 * NB], ADD
                            )
                            for jj in range(2, 8):
                                nc.vector.tensor_tensor(
                                    acc1, acc1, r1[:, jj * NB : (jj + 1) * NB], ADD
                                )
                            nc.sync.dma_start(
                                out=out_sh.ap()[256 + 64 * g : 320 + 64 * g],
                                in_=acc1,
                            )
                if h == 0:
                    nc.gpsimd.collective_compute(
                        "AllToAll",
                        mybir.AluOpType.bypass,
                        ins=[partial0[:]],
                        outs=[a2a0[:]],
                        replica_groups=rgroups,
                    )
                    r0 = red.tile([128, 8 * 2 * NB], bf16, tag="red0", name="red0")
                    for jj in range(8):
                        for t in range(2):
                            nc.sync.dma_start(
                                out=r0[:, (2 * jj + t) * NB : (2 * jj + t + 1) * NB],
                                in_=a2a0.ap()[jj, t],
                            )
                    acc0 = red.tile([128, 2 * NB], f32, tag="acc0", name="acc0")
                    nc.vector.tensor_tensor(
                        acc0, r0[:, 0 : 2 * NB], r0[:, 2 * NB : 4 * NB], ADD
                    )
                    for jj in range(2, 8):
                        nc.vector.tensor_tensor(
                            acc0, acc0, r0[:, 2 * jj * NB : 2 * (jj + 1) * NB], ADD
                        )
                    nc.sync.dma_start(out=out_sh.ap()[0:128], in_=acc0[:, 0:NB])
                    nc.sync.dma_start(out=out_sh.ap()[128:256], in_=acc0[:, NB : 2 * NB])

    nc.finalize()
    return nc


def _get_nc():
    if "nc" not in _CACHE:
        _CACHE["nc"] = _build_nc()
    return _CACHE["nc"]


def kernel(x, W_enc, b_enc, W_dec, b_dec):
    import ml_dtypes
    from concourse.bass_utils import run_bass_kernel_spmd

    bf16 = ml_dtypes.bfloat16
    x = np.asarray(x, dtype=np.float32)
    W_enc = np.asarray(W_enc, dtype=np.float32)
    b_enc = np.asarray(b_enc, dtype=np.float32)
    W_dec = np.asarray(W_dec, dtype=np.float32)
    b_dec = np.asarray(b_dec, dtype=np.float32)

    nc = _get_nc()

    # xT blocked: [h, k, p, c] with xT row k*128+p (= x.reshape(B,LD).T), col h*512+c
    xT = np.ascontiguousarray(
        x.reshape(B, LD).T.reshape(KT, 128, NH, NB).transpose(2, 0, 1, 3)
    ).astype(bf16)
    w_enc_flat = W_enc.reshape(LD, F)
    bdec8 = np.ascontiguousarray(
        (b_dec.reshape(LD) / NCORES).astype(np.float32).reshape(KT, 128).T
    )

    in_maps = []
    for i in range(NCORES):
        fsl = slice(i * FL, (i + 1) * FL)
        we_blk = np.ascontiguousarray(
            w_enc_flat[:, fsl].reshape(KT, 128, FT // 4, NB).transpose(0, 2, 1, 3)
        ).astype(bf16)
        wd_blk = np.ascontiguousarray(
            W_dec[:, fsl, :].reshape(L, FT, 128, 2, NB).transpose(0, 3, 1, 2, 4)
        ).astype(bf16)
        in_maps.append(
            {
                "xT": xT,
                "w_enc": we_blk,
                "w_dec": wd_blk,
                "b_enc": np.ascontiguousarray(b_enc[fsl].reshape(FT, 128).T),
                "b_dec8": bdec8,
            }
        )

    res = run_bass_kernel_spmd(nc, in_maps, list(range(NCORES)))
    _CACHE["last_res"] = res

    xhatT = np.empty((LD, B), dtype=np.float32)
    for i in range(NCORES):
        arr = res.results[i]["out_sh"]  # [512, NB] fp32
        # h0: core i holds partial0 tiles {2i, 2i+1} = ld-tiles 2i,2i+1, cols 0:NB
        xhatT[2 * i * 128 : (2 * i + 2) * 128, 0:NB] = arr[0:256]
        # h1 group g=(l,dg): core i holds ld-tile l*8+dg*4+i//2, partition
        # rows 64*(i%2) : 64*(i%2)+64, cols NB:2NB
        for g in range(4):
            l, dg = divmod(g, 2)
            ld_t = l * 8 + dg * 4 + i // 2
            r0 = ld_t * 128 + 64 * (i % 2)
            xhatT[r0 : r0 + 64, NB : 2 * NB] = arr[256 + 64 * g : 320 + 64 * g]
    return np.ascontiguousarray(xhatT.T).reshape(B, L, D).astype(np.float32)


# revision 5
# speedup vs baseline: 1.1346x; 1.0223x over previous
"""CrossCoder kernel for 8 Trainium2 NeuronCores (Bass/Tile, SPMD).

Math (reference):
    f     = relu(einsum('bld,ldf->bf', x, W_enc) + b_enc)     # [B, F]
    x_hat = einsum('bf,lfd->bld', f, W_dec) + b_dec           # [B, L, D]

Sharding: dict dim F=32768 split 8 ways (FL=4096 per core, tensor parallel
over latents). Each core computes its local f shard (encode) and the
partial decode sum over its latents. Cross-core reduction: bf16 AllToAll
+ on-core DVE sum (A2A moves half the bytes of a fp32 ReduceScatter and
has no CCE-reduce bottleneck). The host reassembles each core's output
slice.

Device layout is feature-major (contraction dim on SBUF partitions);
batch runs in two halves of 512 inside ONE TileContext. Matmul operands
(x, W_enc, W_dec, f) are bf16 (~4e-3 rel err vs 2e-2 tolerance); PSUM
stays fp32. The PE is clamped to 13/16 clock by a board GPIO throttle
(~262ns per N=512 matmul), so the kernel is issue-cadence-bound; every
other engine is arranged to never stall the PE:

- DMA issue costs ~0.6us of engine time each, so weights/x move as
  paired [128,1024] tiles (half the issues) on the sync HWDGE queue,
  everything else (partial stores, reduce loads, outputs) on the scalar
  HWDGE queue.
- Partials are stored p-major ([128, tiles, 512]) so each decode group's
  4 ld-tiles drain as ONE DMA; PSUM drains split vector/scalar.
- Half 0's partial (2MB bf16) is exchanged after half 0, reduce overlaps
  half 1. Half 1 is split into four (l,dg) groups, each AllToAll'd as
  soon as it is written; only the last ~512KB A2A + short reduce is
  exposed at the tail.
- x for half 1 prefetches during half 0's decode; half 0's x interleaves
  with first-group weights so the first matmul issues early.
"""

import numpy as np

B = 1024
L = 2
D = 1024
F = 32768
NCORES = 8
FL = F // NCORES      # 4096 latents per core
LD = L * D            # 2048
KT = LD // 128        # 16 encode k-tiles
KP = KT // 2          # 8 paired encode k-tiles
FT = FL // 128        # 32 f-tiles per core
FP2 = FT // 2         # 16 paired decode f-tiles
NB = 512              # matmul moving free dim
NH = 2                # batch halves

_CACHE = {}


def _build_nc():
    import concourse.bass as bass  # noqa: F401
    import concourse.tile as tile
    from concourse import bacc, mybir

    f32 = mybir.dt.float32
    bf16 = mybir.dt.bfloat16
    ADD = mybir.AluOpType.add
    RELU = mybir.ActivationFunctionType.Relu
    IDENT = mybir.ActivationFunctionType.Identity

    nc = bacc.Bacc()

    # paired tiles: [..., 128, 1024] = two [128,512] tiles side by side
    xT = nc.declare_dram_parameter("xT", [NH, KP, 128, 2 * NB], bf16, isOutput=False)
    w_enc = nc.declare_dram_parameter("w_enc", [FT // 4, KP, 128, 2 * NB], bf16, isOutput=False)
    w_dec = nc.declare_dram_parameter("w_dec", [L, 2, FP2, 128, 2 * NB], bf16, isOutput=False)
    b_enc = nc.declare_dram_parameter("b_enc", [128, FT], f32, isOutput=False)
    b_dec8 = nc.declare_dram_parameter("b_dec8", [128, KT], f32, isOutput=False)
    # out rows: [0:256] = h0 slice (acc0 [128,1024] flat); [256+64g:320+64g]
    # = h1 group g slice (acc1 [128,256] flat). See host remap below.
    out_sh = nc.declare_dram_parameter("out_sh", [512, NB], f32, isOutput=True)

    # partials p-major (bf16): [128, tiles, NB] so one DMA stores a group
    partial0 = nc.dram_tensor("partial0", [128, KT, NB], bf16)
    parts1 = [nc.dram_tensor(f"partial1{g}", [128, 4, NB], bf16) for g in range(4)]
    a2a0 = nc.dram_tensor("a2a0", [8, 16, KT, NB], bf16)
    a2a1 = [nc.dram_tensor(f"a2a1{g}", [8, 16, 4, NB], bf16) for g in range(4)]

    xT_a = xT.ap()
    w_enc_a = w_enc.ap()
    w_dec_a = w_dec.ap()
    rgroups = [list(range(NCORES))]

    with tile.TileContext(nc) as tc:
        with (
            tc.tile_pool(name="xp", bufs=1) as xp,
            tc.tile_pool(name="fp", bufs=1) as fp,
            tc.tile_pool(name="we", bufs=20) as we,
            tc.tile_pool(name="wd", bufs=20) as wd,
            tc.tile_pool(name="stg", bufs=4) as stg,
            tc.tile_pool(name="bias", bufs=1) as bias,
            tc.tile_pool(name="red", bufs=1) as red,
            tc.tile_pool(name="ps", bufs=8, space="PSUM") as ps,
        ):
            benc_t = None
            bdec_t = None

            x_tiles_h = [None, None]
            for h in range(NH):
                x_tiles = x_tiles_h[h]
                if x_tiles is None:
                    x_tiles = []
                    x_tiles_h[h] = x_tiles

                # ---- encode
                f_tiles = []
                for fg in range(FT // 4):
                    pss = [
                        ps.tile([128, NB], f32, tag="ps", name=f"pse{_j}")
                        for _j in range(4)
                    ]
                    for kp in range(KP):
                        if fg == 0 and h == 0:
                            # interleave x pairs with first-group weights
                            xt = xp.tile([128, 2 * NB], bf16, tag=f"x{h}_{kp}", name=f"x{h}_{kp}")
                            nc.sync.dma_start(out=xt, in_=xT_a[h, kp])
                            x_tiles.append(xt)
                        wt = we.tile([128, 2 * NB], bf16, tag="we", name="wet")
                        nc.sync.dma_start(out=wt, in_=w_enc_a[fg, kp])
                        for kin in range(2):
                            k = 2 * kp + kin
                            rhs = x_tiles[kp][:, kin * NB : (kin + 1) * NB]
                            for j in range(4):
                                nc.tensor.matmul(
                                    pss[j],
                                    wt[:, kin * NB + j * 128 : kin * NB + (j + 1) * 128],
                                    rhs,
                                    start=(k == 0),
                                    stop=(k == KT - 1),
                                )
                    if benc_t is None:
                        benc_t = bias.tile([128, FT], f32, name="benc")
                        nc.scalar.dma_start(out=benc_t, in_=b_enc.ap())
                    for j in range(4):
                        ft_idx = fg * 4 + j
                        ftile = fp.tile(
                            [128, NB], bf16, tag=f"f{ft_idx}", name=f"f{ft_idx}"
                        )
                        nc.scalar.activation(
                            ftile,
                            pss[j],
                            RELU,
                            bias=benc_t[:, ft_idx : ft_idx + 1],
                        )
                        f_tiles.append(ftile)

                if h == 0:
                    # prefetch half 1's x during half 0's decode
                    x_tiles_h[1] = []
                    for kp in range(KP):
                        xt = xp.tile([128, 2 * NB], bf16, tag=f"x1_{kp}", name=f"x1_{kp}")
                        nc.sync.dma_start(out=xt, in_=xT_a[1, kp])
                        x_tiles_h[1].append(xt)
                if bdec_t is None:
                    bdec_t = bias.tile([128, KT], f32, name="bdec")
                    nc.scalar.dma_start(out=bdec_t, in_=b_dec8.ap())

                # ---- decode
                for l in range(L):
                    for dg in range(2):
                        g = 2 * l + dg
                        pss = [
                            ps.tile([128, NB], f32, tag="ps", name=f"psd{_j}")
                            for _j in range(4)
                        ]
                        for fkp in range(FP2):
                            wt = wd.tile([128, 2 * NB], bf16, tag="wd", name="wdt")
                            nc.sync.dma_start(out=wt, in_=w_dec_a[l, dg, fkp])
                            for kin in range(2):
                                fk = 2 * fkp + kin
                                for j in range(4):
                                    nc.tensor.matmul(
                                        pss[j],
                                        wt[:, kin * NB + j * 128 : kin * NB + (j + 1) * 128],
                                        f_tiles[fk],
                                        start=(fk == 0),
                                        stop=(fk == FT - 1),
                                    )
                        # drain 4 psum banks into one [128, 4*NB] staging
                        # tile (vector does j=0,1; scalar does j=2,3), then
                        # ONE p-major store
                        stb = stg.tile([128, 4 * NB], bf16, tag="st", name="st")
                        for j in range(4):
                            ld_t = l * 8 + dg * 4 + j
                            dst = stb[:, j * NB : (j + 1) * NB]
                            if j < 2:
                                nc.vector.tensor_scalar_add(
                                    dst, pss[j], bdec_t[:, ld_t : ld_t + 1]
                                )
                            else:
                                nc.scalar.activation(
                                    dst, pss[j], IDENT,
                                    bias=bdec_t[:, ld_t : ld_t + 1],
                                )
                        if h == 0:
                            base = l * 8 + dg * 4
                            nc.scalar.dma_start(
                                out=partial0.ap()[:, base : base + 4, :], in_=stb
                            )
                        else:
                            nc.scalar.dma_start(out=parts1[g].ap()[:], in_=stb)
                            # group complete → exchange + on-core reduce
                            nc.gpsimd.collective_compute(
                                "AllToAll",
                                mybir.AluOpType.bypass,
                                ins=[parts1[g][:]],
                                outs=[a2a1[g][:]],
                                replica_groups=rgroups,
                            )
                            r1 = red.tile([128, 8 * 256], bf16, tag="red1", name="red1")
                            for jj in range(8):
                                eng = nc.sync if jj % 2 == 0 else nc.scalar
                                eng.dma_start(
                                    out=r1[:, jj * 256 : (jj + 1) * 256],
                                    in_=a2a1[g].ap()[jj],
                                )
                            acc1 = red.tile([128, 256], f32, tag=f"acc1_{g}", name=f"acc1_{g}")
                            nc.vector.tensor_tensor(
                                acc1, r1[:, 0:256], r1[:, 256:512], ADD
                            )
                            for jj in range(2, 8):
                                nc.vector.tensor_tensor(
                                    acc1, acc1, r1[:, jj * 256 : (jj + 1) * 256], ADD
                                )
                            nc.scalar.dma_start(
                                out=out_sh.ap()[256 + 64 * g : 320 + 64 * g],
                                in_=acc1,
                            )
                if h == 0:
                    nc.gpsimd.collective_compute(
                        "AllToAll",
                        mybir.AluOpType.bypass,
                        ins=[partial0[:]],
                        outs=[a2a0[:]],
                        replica_groups=rgroups,
                    )
                    r0 = red.tile([128, 8 * 1024], bf16, tag="red0", name="red0")
                    for jj in range(8):
                        eng = nc.sync if jj % 2 == 0 else nc.scalar
                        eng.dma_start(
                            out=r0[:, jj * 1024 : (jj + 1) * 1024],
                            in_=a2a0.ap()[jj],
                        )
                    acc0 = red.tile([128, 1024], f32, tag="acc0", name="acc0")
                    nc.vector.tensor_tensor(
                        acc0, r0[:, 0:1024], r0[:, 1024:2048], ADD
                    )
                    for jj in range(2, 8):
                        nc.vector.tensor_tensor(
                            acc0, acc0, r0[:, jj * 1024 : (jj + 1) * 1024], ADD
                        )
                    nc.scalar.dma_start(out=out_sh.ap()[0:256], in_=acc0)

    nc.finalize()
    return nc


def _get_nc():
    if "nc" not in _CACHE:
        _CACHE["nc"] = _build_nc()
    return _CACHE["nc"]


def kernel(x, W_enc, b_enc, W_dec, b_dec):
    import ml_dtypes
    from concourse.bass_utils import run_bass_kernel_spmd

    bf16 = ml_dtypes.bfloat16
    x = np.asarray(x, dtype=np.float32)
    W_enc = np.asarray(W_enc, dtype=np.float32)
    b_enc = np.asarray(b_enc, dtype=np.float32)
    W_dec = np.asarray(W_dec, dtype=np.float32)
    b_dec = np.asarray(b_dec, dtype=np.float32)

    nc = _get_nc()

    # xT rows = x.reshape(B,LD).T; tile k holds rows k*128..k*128+128,
    # cols h*512..h*512+512; pair kp packs tiles {2kp, 2kp+1} side by side
    xTf = x.reshape(B, LD).T.reshape(KT, 128, NH, NB)          # [k,p,h,c]
    xT = np.ascontiguousarray(
        xTf.reshape(KP, 2, 128, NH, NB).transpose(3, 0, 2, 1, 4).reshape(NH, KP, 128, 2 * NB)
    ).astype(bf16)
    w_enc_flat = W_enc.reshape(LD, F)
    bdec8 = np.ascontiguousarray(
        (b_dec.reshape(LD) / NCORES).astype(np.float32).reshape(KT, 128).T
    )

    in_maps = []
    for i in range(NCORES):
        fsl = slice(i * FL, (i + 1) * FL)
        # [k, p, fg, c] -> pairs over k -> [fg, kp, p, 2c]
        we_t = w_enc_flat[:, fsl].reshape(KT, 128, FT // 4, NB)
        we_blk = np.ascontiguousarray(
            we_t.reshape(KP, 2, 128, FT // 4, NB).transpose(3, 0, 2, 1, 4).reshape(FT // 4, KP, 128, 2 * NB)
        ).astype(bf16)
        # W_dec[l, f, d]: tile (l, dg, fk) = [128 f-rows, 512 d-cols];
        # pair fkp packs {2fkp, 2fkp+1} side by side
        wd_t = W_dec[:, fsl, :].reshape(L, FT, 128, 2, NB)     # [l,fk,p,dg,c]
        wd_blk = np.ascontiguousarray(
            wd_t.reshape(L, FP2, 2, 128, 2, NB).transpose(0, 4, 1, 3, 2, 5).reshape(L, 2, FP2, 128, 2 * NB)
        ).astype(bf16)
        in_maps.append(
            {
                "xT": xT,
                "w_enc": we_blk,
                "w_dec": wd_blk,
                "b_enc": np.ascontiguousarray(b_enc[fsl].reshape(FT, 128).T),
                "b_dec8": bdec8,
            }
        )

    res = run_bass_kernel_spmd(nc, in_maps, list(range(NCORES)))
    _CACHE["last_res"] = res

    # Host reassembly. Partials are p-major [128p, T tiles, 512c]; the A2A
    # hands core i the flat chunk = partitions 16i..16i+16 of every tile.
    #   h0 (T=16): acc0 flat = [16pp, 16t, 512c] -> ld row t*128+16i+pp, col c
    #   h1 group g=(l,dg) (T=4): acc1 flat = [16pp, 4t, 512c]
    #       -> ld row (l*8+dg*4+t)*128 + 16i + pp, col 512+c
    xhatT = np.empty((LD, B), dtype=np.float32)
    xv = xhatT.reshape(KT, 128, B)
    for i in range(NCORES):
        arr = res.results[i]["out_sh"]  # [512, NB] fp32
        h0 = arr[0:256].reshape(16, KT, NB).transpose(1, 0, 2)   # [t, pp, c]
        xv[:, 16 * i : 16 * i + 16, 0:NB] = h0
        for g in range(4):
            l, dg = divmod(g, 2)
            ch = arr[256 + 64 * g : 320 + 64 * g].reshape(16, 4, NB).transpose(1, 0, 2)
            xv[l * 8 + dg * 4 : l * 8 + dg * 4 + 4, 16 * i : 16 * i + 16, NB : 2 * NB] = ch
    return np.ascontiguousarray(xhatT.T).reshape(B, L, D).astype(np.float32)
